# revision 19
# baseline (speedup 1.0000x reference)
"""KANLinear forward on 8 TRN2 NeuronCores.

Reference computes
    out = x @ base_w.T + base_b + spline_w @ linspace(0, 1, S)
The spline branch is batch-independent, so it folds into a single bias
vector on the host. The device kernel is a data-parallel matmul: each
core computes a [2048, 1024] batch shard as out.T tiles ([out-feature
partitions, batch free dim]) so the per-feature bias is a per-partition
scalar add fused into the PSUM->SBUF eviction.

v3 (measured-trace driven):
- fp16 inputs AND outputs (host casts; PSUM accumulates fp32; rel err
  ~4e-4, gate is 2e-2). fp16 runs the PE at the same 1 row/cycle as
  fp32r (231ns/mm measured warm) but halves DMA bytes: loads 6MB,
  stores 4.2MB per core vs 21MB for the f32 baseline.
- The ~6us framework preamble means the first DMA dispatch lands at
  ~7.2us and the fabric (~435GB/s/core, shared by all queues) ramps
  after that. Loads are ordered so the bytes needed first (x0, w01)
  own the early fabric: sync ring carries x0 whole; scalar carries
  w01, w4567, x2; SWDGE carries bias, w23, x1 halves, x3.
- PE warm-up: HAM starts the PE throttled at 1.2GHz and unthrottles
  after ~3.4us of sustained activity. Eight dummy matmuls on a
  memset scratch tile run during the DMA-wait window so the real
  matmul stream starts at full 2.4GHz.
- PSUM pool uses all 8 banks (bufs=8) so psum recycling never gates
  the matmul stream (evictions lag by <=2 groups).
- Stores dispatch round-robin across the three rings in
  eviction-readiness order; the last nb's tail goes out in shrinking
  pieces on separate rings (each store pays ~2us HBM-write receipt).

Layouts (per-partition lines contiguous in DRAM):
  x  -> [NB, 128, KO, 512] fp16  (nb b-tile, ki partition, ko, b col)
  w  -> [MO, 128, KO, 128] fp16  (mo o-tile, ki partition, ko, m col)
  out <- [NB, 128, MO, 512] fp16 (nb, o-partition, mo, b col)
"""

import numpy as np

import concourse.bass as bass  # noqa: F401
import concourse.mybir as mybir
import concourse.tile as tile
from concourse import bacc
from concourse.bass_utils import run_bass_kernel_spmd

B, IN, OUT = 16384, 1024, 1024
N_CORES = 8
BS = B // N_CORES  # 2048 batch rows per core
P = 128  # SBUF partitions
KO = IN // P  # 8 k-subtiles of the contraction dim
MO = OUT // P  # 8 out-feature tiles (psum partition dim)
NB_TILE = 512  # matmul free dim = one fp32 PSUM bank
NB = BS // NB_TILE  # 4 batch tiles per core
N_WARM = 8  # N=512 dummy matmuls to unthrottle the PE
N_WARM_PAD = 30  # N=128 dummy matmuls padding until real data arrives

_CACHE = {}


def _build_nc():
    f32 = mybir.dt.float32
    f16 = mybir.dt.float16

    nc = bacc.Bacc("TRN2", target_bir_lowering=False)
    x_d = nc.dram_tensor("x_t", [NB, P, KO, NB_TILE], f16, kind="ExternalInput")
    w_d = nc.dram_tensor("w_t", [P, MO, KO, P], f16, kind="ExternalInput")
    b_d = nc.dram_tensor("bias_t", [P, MO], f32, kind="ExternalInput")
    o_d = nc.dram_tensor("out_t", [NB, P, MO, NB_TILE], f16, kind="ExternalOutput")

    with tile.TileContext(nc) as tc:
        with (
            tc.tile_pool(name="wp", bufs=1) as wp,
            tc.tile_pool(name="xp", bufs=1) as xp,
            tc.tile_pool(name="cp", bufs=1) as cp,
            tc.tile_pool(name="op", bufs=1) as op,
            tc.tile_pool(name="ps", bufs=7, space="PSUM") as ps,
        ):
            # --- PE warm-up: memset a scratch tile, run dummy matmuls so
            # the HAM clock gate opens while the loads stream in.
            warm_sb = cp.tile([P, NB_TILE], f16)
            nc.vector.memset(warm_sb[:], 0.0)
            warm_ps = ps.tile([P, NB_TILE], f32, tag="warm", bufs=1)
            for _ in range(N_WARM):
                nc.tensor.matmul(
                    warm_ps[:], warm_sb[:, :P], warm_sb[:], start=True, stop=True
                )
            # fine-grained padding: keeps the PE busy (no HAM re-throttle)
            # while delaying the first data-gated matmul by <=107ns
            for _ in range(N_WARM_PAD):
                nc.tensor.matmul(
                    warm_ps[:, :P], warm_sb[:, :P], warm_sb[:, :P],
                    start=True, stop=True,
                )

            bias_sb = cp.tile([P, MO], f32)
            nc.gpsimd.dma_start(bias_sb[:], b_d[:])

            w_sb = [None] * MO
            x_parts = [[] for _ in range(NB)]

            def load_w(mos, engine):
                t = wp.tile([P, len(mos), KO, P], f16, tag=f"w{mos[0]}")
                engine.dma_start(t[:], w_d[:, mos[0] : mos[0] + len(mos)])
                for i, mo in enumerate(mos):
                    w_sb[mo] = t[:, i]

            def load_x(nb, k0, kn, engine):
                t = xp.tile([P, kn, NB_TILE], f16, tag=f"x{nb}_{k0}")
                engine.dma_start(t[:], x_d[nb, :, k0 : k0 + kn])
                x_parts[nb].append((k0, kn, t))

            # Loads: the SDMA engines round-robin queues at packet
            # granularity, so a queue with a big transfer queued first
            # delays other queues' first bytes. First wave is only the
            # immediately-needed small chunks (x0 k01 + w01); the rest
            # follows in PE consumption order.
            load_x(0, 0, 8, nc.sync)
            load_w([0, 1], nc.scalar)
            load_w([4, 5, 6, 7], nc.scalar)
            load_x(2, 0, 8, nc.scalar)
            load_w([2, 3], nc.gpsimd)
            load_x(1, 0, 8, nc.gpsimd)
            load_x(3, 0, 8, nc.gpsimd)

            def x_slice(nb, k):
                for k0, kn, t in x_parts[nb]:
                    if k0 <= k < k0 + kn:
                        return t[:, k - k0]
                raise AssertionError

            # output chunk -> (mo list, engine), round-robin in readiness
            # order; shrinking tail pieces on separate rings.
            out_plan = {
                0: [((0, 1, 2, 3), nc.sync), ((4, 5, 6, 7), nc.gpsimd)],
                1: [((0, 1, 2, 3), nc.scalar), ((4, 5, 6, 7), nc.sync)],
                2: [((0, 1, 2, 3), nc.gpsimd), ((4, 5, 6, 7), nc.scalar)],
                3: [((0, 1, 2, 3), nc.sync), ((4, 5), nc.gpsimd),
                    ((6,), nc.scalar), ((7,), nc.sync)],
            }

            for nb in range(NB):
                chunks = []
                for ci, (mos, eng) in enumerate(out_plan[nb]):
                    t = op.tile([P, len(mos), NB_TILE], f16, tag=f"o{nb}_{ci}",
                                name=f"o{nb}_{ci}")
                    chunks.append((mos, eng, t))
                for mo in range(MO):
                    pt = ps.tile([P, NB_TILE], f32)
                    for k in range(KO):
                        nc.tensor.matmul(
                            pt[:],
                            w_sb[mo][:, k],
                            x_slice(nb, k),
                            start=(k == 0),
                            stop=(k == KO - 1),
                        )
                    for mos, eng, t in chunks:
                        if mo in mos:
                            i = mos.index(mo)
                            nc.vector.tensor_scalar_add(
                                t[:, i], pt[:], bias_sb[:, mo : mo + 1]
                            )
                            if mo == mos[-1]:
                                eng.dma_start(
                                    o_d[nb, :, mos[0] : mos[-1] + 1], t[:]
                                )
                            break

    nc.finalize()
    return nc


def _get_nc():
    if "nc" not in _CACHE:
        _CACHE["nc"] = _build_nc()
    return _CACHE["nc"]


def _prep_inputs(x, base_w, base_b, spline_w):
    x = np.asarray(x, dtype=np.float32)
    base_w = np.asarray(base_w, dtype=np.float32)
    base_b = np.asarray(base_b, dtype=np.float32)
    spline_w = np.asarray(spline_w, dtype=np.float32)

    s_feats = spline_w.shape[1]
    spline_input = np.linspace(0.0, 1.0, s_feats, dtype=np.float32)
    bias = (base_b + spline_w @ spline_input).astype(np.float32)  # [OUT]

    # w_dev[ki, mo, ko, m] = base_w[mo*P + m, ko*P + ki]
    w_dev = np.ascontiguousarray(
        base_w.reshape(MO, P, KO, P).transpose(3, 0, 2, 1).astype(np.float16)
    )
    # bias_dev[p, mo] = bias[mo*P + p]
    bias_dev = np.ascontiguousarray(bias.reshape(MO, P).T)

    x16 = x.astype(np.float16)
    in_maps = []
    for c in range(N_CORES):
        xs = x16[c * BS : (c + 1) * BS]  # [BS, IN]
        # x_dev[nb, ki, ko, col] = xs[nb*NB_TILE + col, ko*P + ki]
        x_dev = np.ascontiguousarray(
            xs.reshape(NB, NB_TILE, KO, P).transpose(0, 3, 2, 1)
        )
        in_maps.append({"x_t": x_dev, "w_t": w_dev, "bias_t": bias_dev})
    return in_maps


def _run(inputs, trace=False, tmpdir=None):
    nc = _get_nc()
    in_maps = _prep_inputs(**inputs)
    res = run_bass_kernel_spmd(
        nc, in_maps, core_ids=list(range(N_CORES)), trace=trace, tmpdir=tmpdir
    )
    outs = []
    for c in range(N_CORES):
        arr = np.asarray(res.results[c]["out_t"])  # [NB, P, MO, NB_TILE] fp16
        # out_core[nb*NB_TILE + col, mo*P + p] = arr[nb, p, mo, col]
        outs.append(
            arr.astype(np.float32).transpose(0, 3, 2, 1).reshape(BS, OUT)
        )
    full = np.ascontiguousarray(np.concatenate(outs, axis=0), dtype=np.float32)
    return full, res


def kernel(**inputs) -> np.ndarray:
    out, _ = _run(inputs, trace=False)
    return out
